# revision 24
# baseline (speedup 1.0000x reference)
"""Trainium2 Bass kernel for nn_DE_89404039234069 (retrieval_knn).

Per batch b (8 batches, one per NeuronCore):
  - context queries i in [0,1024): nearest neighbor among the 1024 context
    points, taken as entry 1 of the stable argsort of |x_i - x_j| (self
    included -> slot 1 of a top-8 sort of negated distances).
  - target queries t (global g = 1024+t): nearest among the causal pool
    j <= g (self included, future masked) -> slot 1 again.
  - y_diff/x_diff/deriv, global BatchNorm over clipped deriv (AllReduce of
    sum/sumsq across the 8 cores), 5 outputs.

Layout notes:
  - distances are computed in [128 queries x L candidates] blocks:
    ACT computes diff = q - x (bias trick), GPSIMD computes
    -|diff| = mult(abs_max(diff,0), -1), DVE max/max_index gives the top-8
    values + occurrence-ordered indices (exactly argsort tie semantics).
  - neighbor values are fetched with gpsimd.ap_gather (x/y replicated on 16
    partitions), idx list round-trips through DRAM to reach the wrapped
    [16 x 128] layout, and results are re-laid out to [128 x 16] via PE
    transposes.
"""

import numpy as np

B = 8
N = 2048
NC_PTS = 1024
P = 128
NBLK = N // P           # 16 query blocks per core
NCTX = NC_PTS // P      # 8 context blocks
EPS_D = 2e-6
BN_EPS = 1e-5
NTOT = B * N            # BatchNorm population size

_CACHE = {}


def _build(n_cores, debug_stats=False, no_collective=False):
    import concourse.bacc as bacc
    import concourse.bass as bass
    import concourse.mybir as mybir
    import concourse.tile as tile
    from concourse.masks import make_identity

    dt = mybir.dt
    f32 = dt.float32
    Alu = mybir.AluOpType
    Act = mybir.ActivationFunctionType

    nc = bacc.Bacc("TRN2", target_bir_lowering=False, debug=False,
                   num_devices=n_cores)

    x_in = nc.dram_tensor("x", [N], f32, kind="ExternalInput").ap()
    y_in = nc.dram_tensor("y", [N], f32, kind="ExternalInput").ap()
    bnw_in = nc.dram_tensor("bn_weight", [1], f32, kind="ExternalInput").ap()
    bnb_in = nc.dram_tensor("bn_bias", [1], f32, kind="ExternalInput").ap()

    ydiff_o = nc.dram_tensor("y_diff", [N], f32, kind="ExternalOutput").ap()
    xdiff_o = nc.dram_tensor("x_diff", [N], f32, kind="ExternalOutput").ap()
    dout_o = nc.dram_tensor("d_out", [2 * N], f32, kind="ExternalOutput").ap()
    xn_o = nc.dram_tensor("x_n", [N], f32, kind="ExternalOutput").ap()
    yn_o = nc.dram_tensor("y_n", [N], f32, kind="ExternalOutput").ap()
    dbg_o = (nc.dram_tensor("dbg", [32], f32, kind="ExternalOutput").ap()
             if debug_stats else None)

    def bcast(ap, n):
        # replicate a flat DRAM row across n partitions (stride-0 read)
        return bass.AP(ap.tensor, ap.offset, [[0, n]] + [list(d) for d in ap.ap])

    with tile.TileContext(nc) as tc:
        with (
            tc.tile_pool(name="const", bufs=1) as constp,
            tc.tile_pool(name="main", bufs=1) as mainp,
            tc.tile_pool(name="work", bufs=2) as workp,
            tc.tile_pool(name="post", bufs=1) as postp,
            tc.tile_pool(name="psum", bufs=3, space="PSUM") as psump,
            tc.tile_pool(name="dram", bufs=1, space="DRAM") as dramp,
        ):
            I128 = constp.tile([P, P], f32)
            make_identity(nc, I128[:])
            ONES = constp.tile([P, 1], f32)
            nc.gpsimd.memset(ONES[:], 1.0)

            # --- replicated candidate rows ---
            X_ALL = mainp.tile([P, N], f32)
            for g in range(4):
                nc.sync.dma_start(X_ALL[32 * g:32 * (g + 1), :], bcast(x_in, 32))
            Y16 = mainp.tile([16, N], f32)
            nc.sync.dma_start(Y16[:], bcast(y_in, 16))

            # --- queries in [128, 16] layout via PE transpose of [16,128] ---
            XW = mainp.tile([16, P], f32)
            nc.sync.dma_start(XW[:], x_in.rearrange("(b f) -> b f", b=16))
            YW = mainp.tile([16, P], f32)
            nc.sync.dma_start(YW[:], y_in.rearrange("(b f) -> b f", b=16))

            XQ = mainp.tile([P, NBLK], f32)
            YQ = mainp.tile([P, NBLK], f32)
            t_xq = psump.tile([P, 16], f32, tag="tp")
            nc.tensor.transpose(t_xq[:], XW[:], I128[0:16, 0:16])
            nc.scalar.copy(XQ[:], t_xq[:])
            t_yq = psump.tile([P, 16], f32, tag="tp")
            nc.tensor.transpose(t_yq[:], YW[:], I128[0:16, 0:16])
            nc.scalar.copy(YQ[:], t_yq[:])

            # --- main argmin loop ---
            M8 = mainp.tile([P, 8 * NBLK], f32)
            I8 = mainp.tile([P, 8 * NBLK], dt.uint32)
            for blk in range(NBLK):
                if blk < NCTX:
                    L = NC_PTS
                    gb = None
                else:
                    gb = NC_PTS + P * (blk - NCTX)
                    L = gb + P
                ABS = workp.tile([P, N], f32, tag="abs")
                NEG = workp.tile([P, N], f32, tag="neg")
                # |x - q| on ACT (bias trick), then exact negation on ACT
                nc.scalar.activation(ABS[:, :L], X_ALL[:, :L], Act.Abs,
                                     bias=XQ[:, blk:blk + 1], scale=-1.0)
                nc.scalar.activation(NEG[:, :L], ABS[:, :L], Act.Copy,
                                     bias=0.0, scale=-1.0)
                if gb is not None:
                    # causal mask: forbid column c > p (j > g); self kept
                    nc.gpsimd.affine_select(
                        out=NEG[:, gb:gb + P], in_=NEG[:, gb:gb + P],
                        compare_op=Alu.is_ge, fill=-1000.0,
                        base=0, pattern=[[-1, P]], channel_multiplier=1)
                nc.vector.max(M8[:, 8 * blk:8 * blk + 8], NEG[:, :L])
                nc.vector.max_index(I8[:, 8 * blk:8 * blk + 8],
                                    M8[:, 8 * blk:8 * blk + 8], NEG[:, :L])

            # --- index slot 1 -> wrapped [16,128] int16 for ap_gather ---
            IDXf32 = postp.tile([P, NBLK], f32)
            i8v = I8[:].rearrange("p (b s) -> p b s", s=8)
            nc.vector.tensor_copy(IDXf32[:], i8v[:, :, 1:2].rearrange("p b o -> p (b o)"))
            t_idx = psump.tile([16, P], f32, tag="tp")
            nc.tensor.transpose(t_idx[:], IDXf32[:], I128[:])
            IDX16 = postp.tile([16, P], dt.int16)
            nc.vector.tensor_copy(IDX16[:], t_idx[:])

            # --- gather neighbor values ---
            # idxs wrap "(s p)": list[k] = IDX16[k%16, k//16] = ix((k%16)*128 + k//16)
            # so gathered free position k holds query q = (k%16)*128 + k//16,
            # i.e. q=(B*128+P) sits at k = P*16 + B.
            XN16R = postp.tile([16, N], f32)
            YN16R = postp.tile([16, N], f32)
            nc.gpsimd.ap_gather(
                out_ap=XN16R[:].rearrange("c (n d) -> c n d", d=1),
                in_ap=X_ALL[0:16, :].rearrange("c (n d) -> c n d", d=1),
                idxs_ap=IDX16[:], channels=16, num_elems=N, d=1, num_idxs=N)
            nc.gpsimd.ap_gather(
                out_ap=YN16R[:].rearrange("c (n d) -> c n d", d=1),
                in_ap=Y16[:].rearrange("c (n d) -> c n d", d=1),
                idxs_ap=IDX16[:], channels=16, num_elems=N, d=1, num_idxs=N)

            # --- re-layout gathered rows to [128, 16] via PE transposes ---
            XN = postp.tile([P, NBLK], f32)
            YN = postp.tile([P, NBLK], f32)
            for c in range(NBLK):
                xsrc = (XN16R[:].rearrange("c (p b) -> c b p", b=16)
                        [:, c:c + 1, :].rearrange("c o p -> c (o p)"))
                t1 = psump.tile([P, 16], f32, tag="tp")
                nc.tensor.transpose(t1[:], xsrc, I128[0:16, 0:16])
                nc.scalar.copy(XN[:, c:c + 1], t1[:, 0:1])
                ysrc = (YN16R[:].rearrange("c (p b) -> c b p", b=16)
                        [:, c:c + 1, :].rearrange("c o p -> c (o p)"))
                t2 = psump.tile([P, 16], f32, tag="tp")
                nc.tensor.transpose(t2[:], ysrc, I128[0:16, 0:16])
                nc.scalar.copy(YN[:, c:c + 1], t2[:, 0:1])

            # --- elementwise tail ---
            XREP = postp.tile([P, NBLK], f32)
            YREP = postp.tile([P, NBLK], f32)
            DEN = postp.tile([P, NBLK], f32)
            D = postp.tile([P, NBLK], f32)
            ABSD = postp.tile([P, NBLK], f32)
            LABEL = postp.tile([P, NBLK], f32)
            D2 = postp.tile([P, NBLK], f32)
            ABSX = postp.tile([P, NBLK], f32)
            nc.vector.tensor_tensor(XREP[:], XQ[:], XN[:], op=Alu.subtract)
            nc.vector.tensor_tensor(YREP[:], YQ[:], YN[:], op=Alu.subtract)
            nc.scalar.activation(ABSX[:], XREP[:], Act.Abs)
            nc.vector.tensor_scalar(DEN[:], ABSX[:], EPS_D, None, op0=Alu.add)
            # d = y/den via reciprocal + one Newton correction on the quotient
            RCP = postp.tile([P, NBLK], f32)
            Q0 = postp.tile([P, NBLK], f32)
            T0 = postp.tile([P, NBLK], f32)
            nc.vector.reciprocal(RCP[:], DEN[:])
            nc.vector.tensor_tensor(Q0[:], YREP[:], RCP[:], op=Alu.mult)
            nc.vector.tensor_tensor(T0[:], DEN[:], Q0[:], op=Alu.mult)
            nc.vector.tensor_tensor(T0[:], YREP[:], T0[:], op=Alu.subtract)
            nc.vector.tensor_tensor(T0[:], RCP[:], T0[:], op=Alu.mult)
            nc.vector.tensor_tensor(D[:], Q0[:], T0[:], op=Alu.add)
            nc.scalar.activation(ABSD[:], D[:], Act.Abs)
            nc.vector.tensor_scalar(LABEL[:], ABSD[:], 200.0, None, op0=Alu.is_le)
            nc.vector.tensor_tensor(D2[:], D[:], LABEL[:], op=Alu.mult)

            # --- BN stats: per-core sum / sumsq -> AllReduce ---
            ST = postp.tile([P, 2], f32)
            SCR = postp.tile([P, NBLK], f32)
            SCR2 = postp.tile([P, NBLK], f32)
            nc.scalar.activation(SCR[:], D2[:], Act.Copy, accum_out=ST[:, 0:1])
            nc.scalar.activation(SCR2[:], D2[:], Act.Square, accum_out=ST[:, 1:2])
            t_s = psump.tile([1, 2], f32, tag="ts")
            nc.tensor.matmul(t_s[:], ONES[:, 0:1], ST[:], start=True, stop=True)
            SSP = postp.tile([1, 128], f32)
            nc.vector.memset(SSP[:], 0.0)
            SS = SSP[:, 0:2]
            nc.scalar.copy(SS, t_s[:])

            cc_in = dramp.tile([128], f32)
            cc_out = dramp.tile([128], f32)
            nc.sync.dma_start(cc_in[:].rearrange("(o f) -> o f", o=1), SSP[:])
            if no_collective:
                nc.sync.dma_start(cc_out[:], cc_in[:])
            else:
                nc.gpsimd.collective_compute(
                    "AllReduce", Alu.add,
                    replica_groups=[list(range(n_cores))],
                    ins=[cc_in[:].opt()], outs=[cc_out[:].opt()])
            GS = postp.tile([1, 2], f32)
            nc.sync.dma_start(GS[:], cc_out[0:2].rearrange("(o f) -> o f", o=1))

            # --- scalar BN math on [1,1] tiles ---
            SC = postp.tile([1, 8], f32)   # mu, ex2, mu2, var, vpe, rs, w, tmp
            nc.vector.tensor_scalar(SC[:, 0:1], GS[:, 0:1], 1.0 / NTOT, None,
                                    op0=Alu.mult)                     # mean
            nc.vector.tensor_scalar(SC[:, 1:2], GS[:, 1:2], 1.0 / NTOT, None,
                                    op0=Alu.mult)                     # E[x^2]
            nc.vector.tensor_tensor(SC[:, 2:3], SC[:, 0:1], SC[:, 0:1],
                                    op=Alu.mult)                      # mean^2
            nc.vector.tensor_tensor(SC[:, 3:4], SC[:, 1:2], SC[:, 2:3],
                                    op=Alu.subtract)                  # var
            # sqrt(var+eps), then reciprocal with one Newton refinement:
            # r' = r * (2 - s*r)
            nc.vector.tensor_scalar(SC[:, 6:7], SC[:, 3:4], BN_EPS, None,
                                    op0=Alu.add)                      # var+eps
            nc.scalar.activation(SC[:, 4:5], SC[:, 6:7], Act.Sqrt)    # s
            nc.vector.reciprocal(SC[:, 5:6], SC[:, 4:5])              # r ~ 1/s
            NT1 = postp.tile([1, 4], f32)
            nc.vector.tensor_tensor(NT1[:, 0:1], SC[:, 4:5], SC[:, 5:6],
                                    op=Alu.mult)                      # s*r
            nc.vector.tensor_scalar(NT1[:, 1:2], NT1[:, 0:1], -1.0, 2.0,
                                    op0=Alu.mult, op1=Alu.add)        # 2 - s*r
            nc.vector.tensor_tensor(NT1[:, 3:4], SC[:, 5:6], NT1[:, 1:2],
                                    op=Alu.mult)                      # r refined

            BWB = postp.tile([1, 2], f32)
            nc.sync.dma_start(BWB[:, 0:1], bnw_in.rearrange("(o f) -> o f", o=1))
            nc.sync.dma_start(BWB[:, 1:2], bnb_in.rearrange("(o f) -> o f", o=1))
            PK = postp.tile([1, 2], f32)
            nc.vector.tensor_tensor(PK[:, 0:1], NT1[:, 3:4], BWB[:, 0:1],
                                    op=Alu.mult)                      # scale = w*rs
            SH1 = postp.tile([1, 1], f32)
            nc.vector.tensor_tensor(SH1[:], SC[:, 0:1], PK[:, 0:1], op=Alu.mult)
            nc.vector.tensor_tensor(PK[:, 1:2], BWB[:, 1:2], SH1[:],
                                    op=Alu.subtract)                  # shift = b - mean*scale
            PKB = postp.tile([P, 2], f32)
            nc.gpsimd.partition_broadcast(PKB[:], PK[:], channels=P)

            DNORM = postp.tile([P, NBLK], f32)
            nc.scalar.activation(DNORM[:], D2[:], Act.Identity,
                                 scale=PKB[:, 0:1], bias=PKB[:, 1:2])

            if dbg_o is not None:
                DB = postp.tile([1, 32], f32)
                nc.vector.tensor_copy(DB[:, 0:2], SS[:])      # local sum/sumsq
                nc.vector.tensor_copy(DB[:, 2:4], GS[:])      # global sum/sumsq
                nc.vector.tensor_copy(DB[:, 4:12], SC[:])     # mu, ex2, mu2, var, s, r, vpe, -
                nc.vector.tensor_copy(DB[:, 12:16], NT1[:])   # newton
                nc.vector.tensor_copy(DB[:, 16:18], BWB[:])   # bn w/b
                nc.vector.tensor_copy(DB[:, 18:20], PK[:])    # scale/shift
                nc.vector.tensor_copy(DB[:, 20:22], PKB[0:1, :])  # bcast row0
                nc.vector.tensor_copy(DB[:, 24:26], ST[0:1, :])  # ST row0
                nc.sync.dma_start(dbg_o.rearrange("(o f) -> o f", o=1), DB[:])

            # --- outputs: transpose [128,16] -> [16,128] and DMA out ---
            def emit(tile_src, dram_ap):
                tp = psump.tile([16, P], f32, tag="tp")
                nc.tensor.transpose(tp[:], tile_src[:], I128[:])
                ob = postp.tile([16, P], f32, tag="ob_" + dram_ap.tensor.name)
                nc.scalar.copy(ob[:], tp[:])
                nc.sync.dma_start(dram_ap.rearrange("(b f) -> b f", b=16), ob[:])

            emit(YREP, ydiff_o)
            emit(XREP, xdiff_o)
            emit(XN, xn_o)
            emit(YN, yn_o)

            DI = postp.tile([16, 2 * P], f32)
            tp_dn = psump.tile([16, P], f32, tag="tp")
            nc.tensor.transpose(tp_dn[:], DNORM[:], I128[:])
            nc.scalar.copy(
                DI[:].rearrange("b (f c) -> b f c", c=2)[:, :, 0:1]
                     .rearrange("b f o -> b (f o)"),
                tp_dn[:])
            tp_lb = psump.tile([16, P], f32, tag="tp")
            nc.tensor.transpose(tp_lb[:], LABEL[:], I128[:])
            nc.scalar.copy(
                DI[:].rearrange("b (f c) -> b f c", c=2)[:, :, 1:2]
                     .rearrange("b f o -> b (f o)"),
                tp_lb[:])
            nc.sync.dma_start(dout_o.rearrange("(b f) -> b f", b=16), DI[:])

    nc.compile()
    return nc


def get_nc(n_cores=8, debug_stats=False):
    key = (n_cores, debug_stats)
    if key not in _CACHE:
        _CACHE[key] = _build(n_cores, debug_stats)
    return _CACHE[key]


def make_in_maps(x, y, bn_weight, bn_bias, n_cores=8):
    x = np.asarray(x, dtype=np.float32).reshape(B, N)
    y = np.asarray(y, dtype=np.float32).reshape(B, N)
    w = np.asarray(bn_weight, dtype=np.float32).reshape(1)
    b = np.asarray(bn_bias, dtype=np.float32).reshape(1)
    return [
        {"x": np.ascontiguousarray(x[i]), "y": np.ascontiguousarray(y[i]),
         "bn_weight": w.copy(), "bn_bias": b.copy()}
        for i in range(n_cores)
    ]


def assemble(results):
    y_diff = np.stack([r["y_diff"] for r in results])[..., None]
    x_diff = np.stack([r["x_diff"] for r in results])[..., None]
    d_out = np.stack([r["d_out"].reshape(N, 2) for r in results])
    x_n = np.stack([r["x_n"] for r in results])[..., None]
    y_n = np.stack([r["y_n"] for r in results])[..., None]
    return (y_diff, x_diff, d_out, x_n, y_n)


_RUNNER = {}


def get_runner(n_cores=B):
    """Build a persistent jitted shard_map runner (compiles once)."""
    if n_cores in _RUNNER:
        return _RUNNER[n_cores]
    import jax
    import numpy as _np
    from jax.sharding import Mesh, PartitionSpec
    from jax.experimental.shard_map import shard_map
    import concourse.mybir as mybir
    from concourse import bass2jax
    from concourse.bass2jax import _bass_exec_p, install_neuronx_cc_hook

    nc = get_nc(n_cores)
    install_neuronx_cc_hook()
    partition_name = (nc.partition_id_tensor.name
                      if nc.partition_id_tensor else None)

    in_names, out_names, out_avals, zero_outs = [], [], [], []
    for alloc in nc.m.functions[0].allocations:
        if not isinstance(alloc, mybir.MemoryLocationSet):
            continue
        name = alloc.memorylocations[0].name
        if alloc.kind == "ExternalInput":
            if name != partition_name:
                in_names.append(name)
        elif alloc.kind == "ExternalOutput":
            shape = tuple(alloc.tensor_shape)
            dtype = mybir.dt.np(alloc.dtype)
            out_names.append(name)
            out_avals.append(jax.core.ShapedArray(shape, dtype))
            zero_outs.append(_np.zeros(shape, dtype))
    n_params = len(in_names)
    n_outs = len(out_avals)
    all_names = in_names + out_names
    if partition_name is not None:
        all_names = all_names + [partition_name]
    donate = tuple(range(n_params, n_params + n_outs))

    def _body(*args):
        operands = list(args)
        if partition_name is not None:
            operands.append(bass2jax.partition_id_tensor())
        outs = _bass_exec_p.bind(
            *operands,
            out_avals=tuple(out_avals),
            in_names=tuple(all_names),
            out_names=tuple(out_names),
            lowering_input_output_aliases=(),
            sim_require_finite=True,
            sim_require_nnan=True,
            nc=nc,
        )
        return tuple(outs)

    devices = jax.devices()[:n_cores]
    mesh = Mesh(_np.asarray(devices), ("core",))
    sharded = jax.jit(
        shard_map(_body, mesh=mesh,
                  in_specs=(PartitionSpec("core"),) * (n_params + n_outs),
                  out_specs=(PartitionSpec("core"),) * n_outs,
                  check_rep=False),
        donate_argnums=donate, keep_unused=True)

    def run(in_maps):
        concat_in = [
            _np.concatenate([_np.atleast_1d(in_maps[c][nm])
                             for c in range(n_cores)], axis=0)
            for nm in in_names
        ]
        concat_zeros = [_np.zeros((n_cores * z.shape[0], *z.shape[1:]), z.dtype)
                        for z in zero_outs]
        out_arrs = sharded(*concat_in, *concat_zeros)
        jax.block_until_ready(out_arrs)
        return [
            {nm: _np.asarray(out_arrs[i]).reshape(n_cores, *out_avals[i].shape)[c]
             for i, nm in enumerate(out_names)}
            for c in range(n_cores)
        ]

    _RUNNER[n_cores] = run
    return run


def kernel(y, x, bn_weight, bn_bias, n_C=1024, n_T=1024):
    assert int(n_C) == NC_PTS and int(n_T) == NC_PTS
    run = get_runner(B)
    in_maps = make_in_maps(x, y, bn_weight, bn_bias, B)
    return assemble(run(in_maps))
